# revision 15
# baseline (speedup 1.0000x reference)
"""Causal attention kernel for Trainium2 (Bass/Tile), 8-core SPMD.

Problem: B=16, S=2048, D=128 fp32 causal attention
    scores = Q @ K^T; scores -= INF*triu(k=1); attn = softmax(scores/sqrt(D));
    out = attn @ V.   Batch dim sharded across 8 cores, 2 batches per core.

Optimizations over the fp32r baseline (77660ns -> ~60500ns):
  - All matmul operands fp16: same 1 col/cycle stream rate as fp32r but no
    4x penalty on <256-col matmuls, and FWL halves LDWEIGHTS (fully hidden;
    512-col matmuls measure at the 216ns ideal).
  - Host packs K^T/Q^T/V-chunked contiguous per 512-col piece: 128x~3KB DMA
    descriptors instead of the baseline V-gather's 4096x512B per batch.
  - exp computes P' = exp(s/sqrt(D) - 2): softmax is shift-invariant (host
    divide cancels the e^-2), and P' <= e^4 keeps fp8e4m3 casts safe.
  - Rowsum l for the full (non-diagonal) chunk groups runs as ONE fp8
    DoubleRow matmul per 2-chunk group (contract 256 rows at 2 fp8/cycle):
    half the PE cycles and a quarter of the instructions of the fp16
    per-chunk version.  All-ones weights make the DR k-interleave order
    irrelevant.  pt -> pt8 casts run on the otherwise-idle DVE.
    Diagonal groups keep exact fp16 rowsums (rows with few attended keys
    can't absorb fp8 numerator/denominator mismatch).
  - Causal masking: instead of PE premask matmuls writing NEG into PSUM,
    GPSIMD affine_select zeroes the mask triangle of P directly in SBUF
    after the exp (keep q >= k), freeing PE cycles and all mask consts.
  - Warmup: 8x 512-col matmuls on a memset dummy lift HAM to K=8/8 while
    the first DMAs land; a dummy exp pre-loads the ACT exp table.

Dataflow per batch / 512-wide q-block / k-chunk group (as v2):
    S^T[k,q] = (K^T chunk).T @ Q^T slice -> fp32 PSUM
    P^T = exp(S^T/sqrt(D) - 2)           -> fp16 SBUF (ACT)
    O^T[d,q] += V_chunk.T @ P^T chunk    -> fp32 PSUM (fp16 PE)
    l[q]     += rowsum(P^T)              -> fp32 PSUM (fp8-DR / fp16 PE)
    host: out = (O^T / l).T  in fp64.
Software pipelining: PV/rowsum of group g is emitted after S+exp of group
g+1; each q-block's evacuation is deferred past the next block's first
group, so the in-order PE queue never stalls on ACT or DVE chains.
"""

import os

os.environ.setdefault("MYCRO_LOCAL_CACHE", "1")

import math

import numpy as np

import concourse.bass as bass
import concourse.mybir as mybir
import concourse.tile as tile
from concourse import bacc
from concourse.bass_utils import run_bass_kernel_spmd

F32 = mybir.dt.float32
F16 = mybir.dt.float16
F8 = mybir.dt.float8e4
EXPF = mybir.ActivationFunctionType.Exp
DR = mybir.MatmulPerfMode.DoubleRow

N_CORES = 8
B = 16
S = 2048
D = 128
BPC = B // N_CORES
SCALE = 1.0 / math.sqrt(float(D))
SHIFT = -2.0  # exp bias; cancels in O/l, keeps P' <= e^4 fp8-safe
NQB = S // 512
NCH = S // 128

USE_DR_ROWSUM = True
USE_GPSIMD_MASK = True

# packed qkv column map, 4 pieces per batch (one per q-block of demand):
#   piece i [1536*i : 1536*(i+1)] = k[512i:512(i+1)] | q[...] | v[...]


def _kt_col(c):
    return 1536 * (c // 512) + (c % 512)


def _qt_col(c):
    return 1536 * (c // 512) + 512 + (c % 512)


def _vr_col(c):
    return 1536 * (c // 512) + 1024 + (c % 512)


def build():
    nc = bacc.Bacc("TRN2", target_bir_lowering=False, debug=False, num_devices=N_CORES)
    x_d = nc.dram_tensor("x", [BPC, 128, 6144], F16, kind="ExternalInput")
    o_d = nc.dram_tensor("o", [BPC, 128, S], F16, kind="ExternalOutput")
    l_d = nc.dram_tensor("l", [1, BPC * NQB * 512], F32, kind="ExternalOutput")

    with tile.TileContext(nc) as tc:
        with (
            tc.tile_pool(name="const", bufs=1) as constp,
            tc.tile_pool(name="qkv", bufs=2) as qkvp,
            tc.tile_pool(name="pt", bufs=6) as ptp,
            tc.tile_pool(name="pt8", bufs=4) as pt8p,
            tc.tile_pool(name="evac", bufs=2) as evacp,
            tc.tile_pool(name="stps", bufs=3, space="PSUM") as stps,
            tc.tile_pool(name="otps", bufs=1, space="PSUM") as otps,
            tc.tile_pool(name="lps", bufs=1, space="PSUM") as lps,
        ):
            # ---- piece-0 load via SWDGE: the GPSIMD engine stream starts
            # ~1.3us before the sync ring's first HWDGE dispatch, so this
            # lands the first q-block's data ~2us earlier.
            qkv_tiles = [
                qkvp.tile([128, 6144], F16, name=f"qkv{b}") for b in range(BPC)
            ]
            nc.gpsimd.dma_start(qkv_tiles[0][:, 0:1536], x_d[0, :, 0:1536])
            # ---- warmup: ACT table load + PE HAM ramp ----
            dummy = constp.tile([128, 512], F16, name="dummy")
            nc.gpsimd.memset(dummy[:], 0.0)
            shiftb = constp.tile([128, 1], F32, name="shiftb")
            nc.gpsimd.memset(shiftb[:], SHIFT)
            warm_exp = constp.tile([128, 1], F32, name="warm_exp")
            nc.scalar.activation(
                warm_exp[:], dummy[:, 0:1], EXPF, bias=shiftb[:], scale=SCALE
            )
            # batch-0 loads, one piece per q-block of demand.  Piece 0 rides
            # the sync ring alone; pieces 1-3 ride the ACT HWDGE ring spaced
            # by short burn activations, so each piece gets near-solo DMA
            # bandwidth and lands just before its q-block needs it.
            # Batch-1 loads are emitted later, behind the first evacuation
            # in the sync stream.
            burn_out = constp.tile([128, 512], F16, name="burn_out")
            for i in range(1, 4):
                with nc.allow_low_precision("ACT burn junk"):
                    nc.scalar.activation(
                        burn_out[:], dummy[:], EXPF, bias=shiftb[:], scale=SCALE
                    )
                nc.scalar.dma_start(
                    qkv_tiles[0][:, 1536 * i : 1536 * (i + 1)],
                    x_d[0, :, 1536 * i : 1536 * (i + 1)],
                )
            warm_ps = stps.tile([128, 512], F32, name="warm_ps", tag="stps")
            for _ in range(6):
                nc.tensor.matmul(
                    warm_ps[:], dummy[:, 0:128], dummy[:], start=True, stop=True
                )

            # ---- consts ----
            ones_h = constp.tile([128, 128], F16, name="ones_h")
            nc.gpsimd.memset(ones_h[:], 1.0)
            lsb_all = constp.tile([1, BPC * NQB * 512], F32, name="lsb_all")
            if USE_DR_ROWSUM:
                ones8 = constp.tile([128, 256], F8, name="ones8")
                nc.gpsimd.memset(ones8[:], 1.0)
            if not USE_GPSIMD_MASK:
                from concourse.masks import make_identity

                NEGC = -57344.0
                ident = constp.tile([128, 128], F32, name="ident")
                make_identity(nc, ident[:])
                u01_f = constp.tile([128, 128], F32, name="u01_f")
                nc.gpsimd.memset(u01_f[:], 1.0)
                nc.gpsimd.affine_select(
                    out=u01_f[:],
                    in_=u01_f[:],
                    compare_op=mybir.AluOpType.is_ge,
                    fill=0.0,
                    base=-1,
                    pattern=[[1, 128]],
                    channel_multiplier=-1,
                )
                u01_h = constp.tile([128, 128], F16, name="u01_h")
                idneg_h = constp.tile([128, 128], F16, name="idneg_h")
                idneg2_h = constp.tile([128, 384], F16, name="idneg2_h")
                with nc.allow_low_precision("fp16 mask consts are exact"):
                    nc.vector.tensor_copy(u01_h[:], u01_f[:])
                    nc.vector.tensor_scalar_mul(idneg_h[:], ident[:], NEGC)
                    nc.gpsimd.memset(idneg2_h[:], 0.0)
                    nc.vector.tensor_scalar_mul(idneg2_h[:, 0:128], ident[:], NEGC)
                    nc.vector.tensor_scalar_mul(idneg2_h[:, 256:384], ident[:], NEGC)

            pending_pv = [None]
            pending_evac = [None]

            def flush_pv():
                if pending_pv[0] is not None:
                    pending_pv[0]()
                    pending_pv[0] = None

            def flush_evac():
                if pending_evac[0] is not None:
                    pending_evac[0]()
                    pending_evac[0] = None

            for b in range(BPC):
                qkv = qkv_tiles[b]

                def kt_ap(j, qkv=qkv):
                    c = _kt_col(j * 128)
                    return qkv[:, c : c + 128]

                def qt_ap(c0, w, qkv=qkv):
                    c = _qt_col(c0)
                    return qkv[:, c : c + w]

                def vr_ap(j, qkv=qkv):
                    c = _vr_col(j * 128)
                    return qkv[:, c : c + 128]

                for qb in range(NQB):
                    if qb == 2 and b + 1 < BPC:
                        # emitted after qb0's evac dma in the sync stream:
                        # the sync sequencer blocks on that evac's readiness,
                        # so these transfers don't steal bandwidth from the
                        # critical first-batch pieces.
                        nxt = qkv_tiles[b + 1]
                        nc.sync.dma_start(nxt[:, 0:3072], x_d[b + 1, :, 0:3072])
                        nc.sync.dma_start(
                            nxt[:, 3072:6144], x_d[b + 1, :, 3072:6144]
                        )
                    n_full = 4 * qb
                    n_ch = n_full + 4
                    q0 = qb * 512

                    # (chunks, extent, is_diag); chunk = (j, qoff, width, col)
                    groups = []
                    jf = 0
                    while jf < n_full:
                        g = min(2, n_full - jf)
                        groups.append(
                            (
                                [(jf + c, 0, 512, c * 512) for c in range(g)],
                                g * 512,
                                False,
                            )
                        )
                        jf += g
                    groups.append(
                        ([(n_full, 0, 512, 0), (n_full + 1, 128, 384, 512)], 896, True)
                    )
                    groups.append(
                        (
                            [(n_full + 2, 256, 256, 0), (n_full + 3, 384, 128, 256)],
                            384,
                            True,
                        )
                    )

                    ot = otps.tile([128, 512], F32, name="ot")
                    lp = lps.tile([128, 512], F32, name="lp", tag="lp")

                    for gi, (chunks, extent, is_diag) in enumerate(groups):
                        st = stps.tile([128, 1024], F32, name="st", tag="stps")
                        if not USE_GPSIMD_MASK and is_diag:
                            if chunks[0][3] == 0 and chunks[1][3] == 256:
                                nc.tensor.matmul(
                                    st[:, 0:384],
                                    u01_h[:],
                                    idneg2_h[:],
                                    start=True,
                                    stop=False,
                                )
                                premasked = True
                            else:
                                premasked = False
                        else:
                            premasked = True  # no PE premask needed
                        for (j, qoff, width, col) in chunks:
                            qk_start = True
                            if not USE_GPSIMD_MASK and is_diag:
                                if not premasked:
                                    nc.tensor.matmul(
                                        st[:, col : col + 128],
                                        u01_h[:],
                                        idneg_h[:],
                                        start=True,
                                        stop=False,
                                    )
                                qk_start = False
                            nc.tensor.matmul(
                                st[:, col : col + width],
                                kt_ap(j),
                                qt_ap(q0 + qoff, width),
                                start=qk_start,
                                stop=True,
                            )
                        pt = ptp.tile([128, 1024], F16, name="pt", tag="pt")
                        with nc.allow_low_precision("fp16 P within tolerance"):
                            nc.scalar.activation(
                                pt[:, 0:extent],
                                st[:, 0:extent],
                                EXPF,
                                bias=shiftb[:],
                                scale=SCALE,
                            )
                        if USE_GPSIMD_MASK and is_diag:
                            # zero P where q < k (keep local col >= partition)
                            for (j, qoff, width, col) in chunks:
                                nc.gpsimd.affine_select(
                                    out=pt[:, col : col + 128],
                                    in_=pt[:, col : col + 128],
                                    compare_op=mybir.AluOpType.is_ge,
                                    fill=0.0,
                                    base=0,
                                    pattern=[[1, 128]],
                                    channel_multiplier=-1,
                                )
                        pt8 = None
                        if USE_DR_ROWSUM and not is_diag and len(chunks) == 2:
                            pt8 = pt8p.tile([128, 1024], F8, name="pt8", tag="pt8")
                            with nc.allow_low_precision("fp8 rowsum only"):
                                nc.vector.tensor_copy(pt8[:], pt[:])
                        flush_pv()
                        if gi == 1:
                            flush_evac()

                        def pv(
                            chunks=chunks,
                            ot=ot,
                            lp=lp,
                            pt=pt,
                            pt8=pt8,
                            n_ch=n_ch,
                            is_last=(gi == len(groups) - 1),
                            b=b,
                            q0=q0,
                            qb=qb,
                            vr_ap=vr_ap,
                        ):
                            final = (
                                is_last and b == BPC - 1 and qb == NQB - 1
                            )
                            if not final:
                                for (j, qoff, width, col) in chunks:
                                    nc.tensor.matmul(
                                        ot[:, qoff : qoff + width],
                                        vr_ap(j),
                                        pt[:, col : col + width],
                                        start=(j == 0),
                                        stop=(j == n_ch - 1),
                                    )
                            if pt8 is not None:
                                j0 = chunks[0][0]
                                nc.tensor.matmul(
                                    lp[:],
                                    ones8[:].rearrange("p (i m) -> p i m", i=2),
                                    pt8[:].rearrange("p (i n) -> p i n", i=2),
                                    start=(j0 == 0),
                                    stop=(j0 + 1 == n_ch - 1),
                                    perf_mode=DR,
                                )
                            else:
                                for (j, qoff, width, col) in chunks:
                                    nc.tensor.matmul(
                                        lp[:, qoff : qoff + width],
                                        ones_h[:],
                                        pt[:, col : col + width],
                                        start=(j == 0),
                                        stop=(j == n_ch - 1),
                                    )
                            if final:
                                # rowsums emitted above; PV matmuls now so
                                # the l evacuation overlaps them
                                for (j, qoff, width, col) in chunks:
                                    nc.tensor.matmul(
                                        ot[:, qoff : qoff + width],
                                        vr_ap(j),
                                        pt[:, col : col + width],
                                        start=(j == 0),
                                        stop=(j == n_ch - 1),
                                    )
                            if not is_last:
                                return
                            ots = evacp.tile([128, 512], F16, name="ots")
                            li = (b * NQB + qb) * 512
                            last = b == BPC - 1 and qb == NQB - 1
                            if last:
                                # ACT is idle after its last exp: route the
                                # rowsum row and the first output half through
                                # it so they start at PE-end, in parallel with
                                # the DVE's cast/copy backlog.
                                nc.scalar.copy(
                                    lsb_all[:, li : li + 512], lp[0:1, :]
                                )
                                nc.sync.dma_start(l_d[:], lsb_all[:])
                                with nc.allow_low_precision("fp16 O^T ship"):
                                    nc.scalar.copy(ots[:, 0:256], ot[:, 0:256])
                                    nc.sync.dma_start(
                                        o_d[b, :, q0 : q0 + 256], ots[:, 0:256]
                                    )
                                    nc.vector.tensor_copy(
                                        ots[:, 256:512], ot[:, 256:512]
                                    )
                                    nc.sync.dma_start(
                                        o_d[b, :, q0 + 256 : q0 + 512],
                                        ots[:, 256:512],
                                    )
                            else:
                                nc.vector.tensor_copy(
                                    lsb_all[:, li : li + 512], lp[0:1, :]
                                )
                                with nc.allow_low_precision("fp16 O^T ship"):
                                    nc.vector.tensor_copy(ots[:], ot[:])

                            def evac(b=b, q0=q0, qb=qb, ots=ots):
                                nc.sync.dma_start(o_d[b, :, q0 : q0 + 512], ots[:])

                            if not last:
                                pending_evac[0] = evac

                        pending_pv[0] = pv

            flush_pv()
            flush_evac()
    nc.compile()
    return nc


_NC_CACHE = None


def _get_nc():
    global _NC_CACHE
    if _NC_CACHE is None:
        _NC_CACHE = build()
    return _NC_CACHE


def kernel(query, key, value, _trace=False):
    nc = _get_nc()
    in_maps = []
    for c in range(N_CORES):
        sl = slice(c * BPC, (c + 1) * BPC)
        q = np.asarray(query[sl], dtype=np.float32).astype(np.float16)
        k = np.asarray(key[sl], dtype=np.float32).astype(np.float16)
        v = np.asarray(value[sl], dtype=np.float32).astype(np.float16)
        x = np.empty((BPC, 128, 6144), np.float16)
        for b in range(BPC):
            ktp = k[b].T
            qtp = q[b].T
            vrp = np.ascontiguousarray(
                v[b].reshape(16, 128, 128).transpose(1, 0, 2)
            ).reshape(128, 2048)
            for i in range(4):
                base = 1536 * i
                cs = slice(512 * i, 512 * (i + 1))
                x[b, :, base : base + 512] = ktp[:, cs]
                x[b, :, base + 512 : base + 1024] = qtp[:, cs]
                x[b, :, base + 1024 : base + 1536] = vrp[:, cs]
        in_maps.append({"x": x})
    res = run_bass_kernel_spmd(
        nc, in_maps, core_ids=list(range(N_CORES)), trace=_trace
    )
    outs = []
    for c in range(N_CORES):
        o = res.results[c]["o"].astype(np.float32)
        l = res.results[c]["l"].reshape(BPC, S).astype(np.float32)
        outs.append(o.transpose(0, 2, 1) / l[:, :, None])
    out = np.ascontiguousarray(np.concatenate(outs, axis=0), dtype=np.float32)
    if _trace:
        return out, res
    return out


# revision 16
# speedup vs baseline: 1.2886x; 1.2886x over previous
"""Causal attention kernel for Trainium2 (Bass/Tile), 8-core SPMD.

Problem: B=16, S=2048, D=128 fp32 causal attention
    scores = Q @ K^T; scores -= INF*triu(k=1); attn = softmax(scores/sqrt(D));
    out = attn @ V.   Batch dim sharded across 8 cores, 2 batches per core.

Optimizations over the fp32r baseline (77660ns -> ~60500ns):
  - All matmul operands fp16: same 1 col/cycle stream rate as fp32r but no
    4x penalty on <256-col matmuls, and FWL halves LDWEIGHTS (fully hidden;
    512-col matmuls measure at the 216ns ideal).
  - Host packs K^T/Q^T/V-chunked contiguous per 512-col piece: 128x~3KB DMA
    descriptors instead of the baseline V-gather's 4096x512B per batch.
  - exp computes P' = exp(s/sqrt(D) - 2): softmax is shift-invariant (host
    divide cancels the e^-2), and P' <= e^4 keeps fp8e4m3 casts safe.
  - Rowsum l for the full (non-diagonal) chunk groups runs as ONE fp8
    DoubleRow matmul per 2-chunk group (contract 256 rows at 2 fp8/cycle):
    half the PE cycles and a quarter of the instructions of the fp16
    per-chunk version.  All-ones weights make the DR k-interleave order
    irrelevant.  pt -> pt8 casts run on the otherwise-idle DVE.
    Diagonal groups keep exact fp16 rowsums (rows with few attended keys
    can't absorb fp8 numerator/denominator mismatch).
  - Causal masking: instead of PE premask matmuls writing NEG into PSUM,
    GPSIMD affine_select zeroes the mask triangle of P directly in SBUF
    after the exp (keep q >= k), freeing PE cycles and all mask consts.
  - Warmup: 8x 512-col matmuls on a memset dummy lift HAM to K=8/8 while
    the first DMAs land; a dummy exp pre-loads the ACT exp table.

Dataflow per batch / 512-wide q-block / k-chunk group (as v2):
    S^T[k,q] = (K^T chunk).T @ Q^T slice -> fp32 PSUM
    P^T = exp(S^T/sqrt(D) - 2)           -> fp16 SBUF (ACT)
    O^T[d,q] += V_chunk.T @ P^T chunk    -> fp32 PSUM (fp16 PE)
    l[q]     += rowsum(P^T)              -> fp32 PSUM (fp8-DR / fp16 PE)
    host: out = (O^T / l).T  in fp64.
Software pipelining: PV/rowsum of group g is emitted after S+exp of group
g+1; each q-block's evacuation is deferred past the next block's first
group, so the in-order PE queue never stalls on ACT or DVE chains.
"""

import os

os.environ.setdefault("MYCRO_LOCAL_CACHE", "1")

import math

import numpy as np

import concourse.bass as bass
import concourse.mybir as mybir
import concourse.tile as tile
from concourse import bacc
from concourse.bass_utils import run_bass_kernel_spmd

F32 = mybir.dt.float32
F16 = mybir.dt.float16
F8 = mybir.dt.float8e4
EXPF = mybir.ActivationFunctionType.Exp
DR = mybir.MatmulPerfMode.DoubleRow

N_CORES = 8
B = 16
S = 2048
D = 128
BPC = B // N_CORES
SCALE = 1.0 / math.sqrt(float(D))
SHIFT = -2.0  # exp bias; cancels in O/l, keeps P' <= e^4 fp8-safe
NQB = S // 512
NCH = S // 128

USE_DR_ROWSUM = True
USE_GPSIMD_MASK = True

# packed qkv column map, 4 pieces per batch (one per q-block of demand):
#   piece i [1536*i : 1536*(i+1)] = k[512i:512(i+1)] | q[...] | v[...]


def _kt_col(c):
    return 1536 * (c // 512) + (c % 512)


def _qt_col(c):
    return 1536 * (c // 512) + 512 + (c % 512)


def _vr_col(c):
    return 1536 * (c // 512) + 1024 + (c % 512)


def build():
    nc = bacc.Bacc("TRN2", target_bir_lowering=False, debug=False, num_devices=N_CORES)
    x_d = nc.dram_tensor("x", [BPC, 128, 6144], F16, kind="ExternalInput")
    o_d = nc.dram_tensor("o", [BPC, 128, S], F16, kind="ExternalOutput")
    l_d = nc.dram_tensor("l", [1, BPC * NQB * 512], F32, kind="ExternalOutput")

    with tile.TileContext(nc) as tc:
        with (
            tc.tile_pool(name="const", bufs=1) as constp,
            tc.tile_pool(name="qkv", bufs=2) as qkvp,
            tc.tile_pool(name="pt", bufs=6) as ptp,
            tc.tile_pool(name="pt8", bufs=4) as pt8p,
            tc.tile_pool(name="evac", bufs=2) as evacp,
            tc.tile_pool(name="stps", bufs=3, space="PSUM") as stps,
            tc.tile_pool(name="otps", bufs=1, space="PSUM") as otps,
            tc.tile_pool(name="lps", bufs=1, space="PSUM") as lps,
        ):
            # ---- warmup: ACT table load + PE HAM ramp, before anything ----
            dummy = constp.tile([128, 512], F16, name="dummy")
            nc.gpsimd.memset(dummy[:], 0.0)
            shiftb = constp.tile([128, 1], F32, name="shiftb")
            nc.gpsimd.memset(shiftb[:], SHIFT)
            warm_exp = constp.tile([128, 1], F32, name="warm_exp")
            nc.scalar.activation(
                warm_exp[:], dummy[:, 0:1], EXPF, bias=shiftb[:], scale=SCALE
            )
            # batch-0 loads, one piece per q-block of demand.  Piece 0 rides
            # the sync ring alone; pieces 1-3 ride the ACT HWDGE ring spaced
            # by short burn activations, so each piece gets near-solo DMA
            # bandwidth and lands just before its q-block needs it.
            # Batch-1 loads are emitted later, behind the first evacuation
            # in the sync stream.
            qkv_tiles = [
                qkvp.tile([128, 6144], F16, name=f"qkv{b}") for b in range(BPC)
            ]
            burn_out = constp.tile([128, 512], F16, name="burn_out")
            nc.sync.dma_start(qkv_tiles[0][:, 0:1536], x_d[0, :, 0:1536])
            for i in range(1, 4):
                with nc.allow_low_precision("ACT burn junk"):
                    nc.scalar.activation(
                        burn_out[:], dummy[:], EXPF, bias=shiftb[:], scale=SCALE
                    )
                nc.scalar.dma_start(
                    qkv_tiles[0][:, 1536 * i : 1536 * (i + 1)],
                    x_d[0, :, 1536 * i : 1536 * (i + 1)],
                )
            warm_ps = stps.tile([128, 512], F32, name="warm_ps", tag="stps")
            for _ in range(8):
                nc.tensor.matmul(
                    warm_ps[:], dummy[:, 0:128], dummy[:], start=True, stop=True
                )

            # ---- consts ----
            ones_h = constp.tile([128, 128], F16, name="ones_h")
            nc.gpsimd.memset(ones_h[:], 1.0)
            lsb_all = constp.tile([1, BPC * NQB * 512], F32, name="lsb_all")
            if USE_DR_ROWSUM:
                ones8 = constp.tile([128, 256], F8, name="ones8")
                nc.gpsimd.memset(ones8[:], 1.0)
            if not USE_GPSIMD_MASK:
                from concourse.masks import make_identity

                NEGC = -57344.0
                ident = constp.tile([128, 128], F32, name="ident")
                make_identity(nc, ident[:])
                u01_f = constp.tile([128, 128], F32, name="u01_f")
                nc.gpsimd.memset(u01_f[:], 1.0)
                nc.gpsimd.affine_select(
                    out=u01_f[:],
                    in_=u01_f[:],
                    compare_op=mybir.AluOpType.is_ge,
                    fill=0.0,
                    base=-1,
                    pattern=[[1, 128]],
                    channel_multiplier=-1,
                )
                u01_h = constp.tile([128, 128], F16, name="u01_h")
                idneg_h = constp.tile([128, 128], F16, name="idneg_h")
                idneg2_h = constp.tile([128, 384], F16, name="idneg2_h")
                with nc.allow_low_precision("fp16 mask consts are exact"):
                    nc.vector.tensor_copy(u01_h[:], u01_f[:])
                    nc.vector.tensor_scalar_mul(idneg_h[:], ident[:], NEGC)
                    nc.gpsimd.memset(idneg2_h[:], 0.0)
                    nc.vector.tensor_scalar_mul(idneg2_h[:, 0:128], ident[:], NEGC)
                    nc.vector.tensor_scalar_mul(idneg2_h[:, 256:384], ident[:], NEGC)

            pending_pv = [None]
            pending_evac = [None]

            def flush_pv():
                if pending_pv[0] is not None:
                    pending_pv[0]()
                    pending_pv[0] = None

            def flush_evac():
                if pending_evac[0] is not None:
                    pending_evac[0]()
                    pending_evac[0] = None

            for b in range(BPC):
                qkv = qkv_tiles[b]

                def kt_ap(j, qkv=qkv):
                    c = _kt_col(j * 128)
                    return qkv[:, c : c + 128]

                def qt_ap(c0, w, qkv=qkv):
                    c = _qt_col(c0)
                    return qkv[:, c : c + w]

                def vr_ap(j, qkv=qkv):
                    c = _vr_col(j * 128)
                    return qkv[:, c : c + 128]

                for qb in range(NQB):
                    if qb == 2 and b + 1 < BPC:
                        # emitted after qb0's evac dma in the sync stream:
                        # the sync sequencer blocks on that evac's readiness,
                        # so these transfers don't steal bandwidth from the
                        # critical first-batch pieces.
                        nxt = qkv_tiles[b + 1]
                        nc.sync.dma_start(nxt[:, 0:3072], x_d[b + 1, :, 0:3072])
                        nc.sync.dma_start(
                            nxt[:, 3072:6144], x_d[b + 1, :, 3072:6144]
                        )
                    n_full = 4 * qb
                    n_ch = n_full + 4
                    q0 = qb * 512

                    # (chunks, extent, is_diag); chunk = (j, qoff, width, col)
                    groups = []
                    jf = 0
                    while jf < n_full:
                        g = min(2, n_full - jf)
                        groups.append(
                            (
                                [(jf + c, 0, 512, c * 512) for c in range(g)],
                                g * 512,
                                False,
                            )
                        )
                        jf += g
                    groups.append(
                        ([(n_full, 0, 512, 0), (n_full + 1, 128, 384, 512)], 896, True)
                    )
                    groups.append(
                        (
                            [(n_full + 2, 256, 256, 0), (n_full + 3, 384, 128, 256)],
                            384,
                            True,
                        )
                    )

                    ot = otps.tile([128, 512], F32, name="ot")
                    lp = lps.tile([128, 512], F32, name="lp", tag="lp")

                    for gi, (chunks, extent, is_diag) in enumerate(groups):
                        st = stps.tile([128, 1024], F32, name="st", tag="stps")
                        if not USE_GPSIMD_MASK and is_diag:
                            if chunks[0][3] == 0 and chunks[1][3] == 256:
                                nc.tensor.matmul(
                                    st[:, 0:384],
                                    u01_h[:],
                                    idneg2_h[:],
                                    start=True,
                                    stop=False,
                                )
                                premasked = True
                            else:
                                premasked = False
                        else:
                            premasked = True  # no PE premask needed
                        for (j, qoff, width, col) in chunks:
                            qk_start = True
                            if not USE_GPSIMD_MASK and is_diag:
                                if not premasked:
                                    nc.tensor.matmul(
                                        st[:, col : col + 128],
                                        u01_h[:],
                                        idneg_h[:],
                                        start=True,
                                        stop=False,
                                    )
                                qk_start = False
                            nc.tensor.matmul(
                                st[:, col : col + width],
                                kt_ap(j),
                                qt_ap(q0 + qoff, width),
                                start=qk_start,
                                stop=True,
                            )
                        pt = ptp.tile([128, 1024], F16, name="pt", tag="pt")
                        with nc.allow_low_precision("fp16 P within tolerance"):
                            nc.scalar.activation(
                                pt[:, 0:extent],
                                st[:, 0:extent],
                                EXPF,
                                bias=shiftb[:],
                                scale=SCALE,
                            )
                        if USE_GPSIMD_MASK and is_diag:
                            # zero P where q < k (keep local col >= partition)
                            for (j, qoff, width, col) in chunks:
                                nc.gpsimd.affine_select(
                                    out=pt[:, col : col + 128],
                                    in_=pt[:, col : col + 128],
                                    compare_op=mybir.AluOpType.is_ge,
                                    fill=0.0,
                                    base=0,
                                    pattern=[[1, 128]],
                                    channel_multiplier=-1,
                                )
                        pt8 = None
                        if USE_DR_ROWSUM and not is_diag and len(chunks) == 2:
                            pt8 = pt8p.tile([128, 1024], F8, name="pt8", tag="pt8")
                            with nc.allow_low_precision("fp8 rowsum only"):
                                nc.vector.tensor_copy(pt8[:], pt[:])
                        flush_pv()
                        if gi == 1:
                            flush_evac()

                        def pv(
                            chunks=chunks,
                            ot=ot,
                            lp=lp,
                            pt=pt,
                            pt8=pt8,
                            n_ch=n_ch,
                            is_last=(gi == len(groups) - 1),
                            b=b,
                            q0=q0,
                            qb=qb,
                            vr_ap=vr_ap,
                        ):
                            final = (
                                is_last and b == BPC - 1 and qb == NQB - 1
                            )
                            if not final:
                                for (j, qoff, width, col) in chunks:
                                    nc.tensor.matmul(
                                        ot[:, qoff : qoff + width],
                                        vr_ap(j),
                                        pt[:, col : col + width],
                                        start=(j == 0),
                                        stop=(j == n_ch - 1),
                                    )
                            if pt8 is not None:
                                j0 = chunks[0][0]
                                nc.tensor.matmul(
                                    lp[:],
                                    ones8[:].rearrange("p (i m) -> p i m", i=2),
                                    pt8[:].rearrange("p (i n) -> p i n", i=2),
                                    start=(j0 == 0),
                                    stop=(j0 + 1 == n_ch - 1),
                                    perf_mode=DR,
                                )
                            else:
                                for (j, qoff, width, col) in chunks:
                                    nc.tensor.matmul(
                                        lp[:, qoff : qoff + width],
                                        ones_h[:],
                                        pt[:, col : col + width],
                                        start=(j == 0),
                                        stop=(j == n_ch - 1),
                                    )
                            if final:
                                # rowsums emitted above; PV matmuls now so
                                # the l evacuation overlaps them
                                for (j, qoff, width, col) in chunks:
                                    nc.tensor.matmul(
                                        ot[:, qoff : qoff + width],
                                        vr_ap(j),
                                        pt[:, col : col + width],
                                        start=(j == 0),
                                        stop=(j == n_ch - 1),
                                    )
                            if not is_last:
                                return
                            ots = evacp.tile([128, 512], F16, name="ots")
                            li = (b * NQB + qb) * 512
                            last = b == BPC - 1 and qb == NQB - 1
                            if last:
                                # ACT is idle after its last exp: route the
                                # rowsum row and the first output half through
                                # it so they start at PE-end, in parallel with
                                # the DVE's cast/copy backlog.
                                nc.scalar.copy(
                                    lsb_all[:, li : li + 512], lp[0:1, :]
                                )
                                nc.sync.dma_start(l_d[:], lsb_all[:])
                                with nc.allow_low_precision("fp16 O^T ship"):
                                    nc.scalar.copy(ots[:, 0:256], ot[:, 0:256])
                                    nc.sync.dma_start(
                                        o_d[b, :, q0 : q0 + 256], ots[:, 0:256]
                                    )
                                    nc.vector.tensor_copy(
                                        ots[:, 256:512], ot[:, 256:512]
                                    )
                                    nc.sync.dma_start(
                                        o_d[b, :, q0 + 256 : q0 + 512],
                                        ots[:, 256:512],
                                    )
                            else:
                                nc.vector.tensor_copy(
                                    lsb_all[:, li : li + 512], lp[0:1, :]
                                )
                                with nc.allow_low_precision("fp16 O^T ship"):
                                    nc.vector.tensor_copy(ots[:], ot[:])

                            def evac(b=b, q0=q0, qb=qb, ots=ots):
                                nc.sync.dma_start(o_d[b, :, q0 : q0 + 512], ots[:])

                            if not last:
                                pending_evac[0] = evac

                        pending_pv[0] = pv

            flush_pv()
            flush_evac()
    nc.compile()
    return nc


_NC_CACHE = None


def _get_nc():
    global _NC_CACHE
    if _NC_CACHE is None:
        _NC_CACHE = build()
    return _NC_CACHE


def kernel(query, key, value, _trace=False):
    nc = _get_nc()
    in_maps = []
    for c in range(N_CORES):
        sl = slice(c * BPC, (c + 1) * BPC)
        q = np.asarray(query[sl], dtype=np.float32).astype(np.float16)
        k = np.asarray(key[sl], dtype=np.float32).astype(np.float16)
        v = np.asarray(value[sl], dtype=np.float32).astype(np.float16)
        x = np.empty((BPC, 128, 6144), np.float16)
        for b in range(BPC):
            ktp = k[b].T
            qtp = q[b].T
            vrp = np.ascontiguousarray(
                v[b].reshape(16, 128, 128).transpose(1, 0, 2)
            ).reshape(128, 2048)
            for i in range(4):
                base = 1536 * i
                cs = slice(512 * i, 512 * (i + 1))
                x[b, :, base : base + 512] = ktp[:, cs]
                x[b, :, base + 512 : base + 1024] = qtp[:, cs]
                x[b, :, base + 1024 : base + 1536] = vrp[:, cs]
        in_maps.append({"x": x})
    res = run_bass_kernel_spmd(
        nc, in_maps, core_ids=list(range(N_CORES)), trace=_trace
    )
    outs = []
    for c in range(N_CORES):
        o = res.results[c]["o"].astype(np.float32)
        l = res.results[c]["l"].reshape(BPC, S).astype(np.float32)
        outs.append(o.transpose(0, 2, 1) / l[:, :, None])
    out = np.ascontiguousarray(np.concatenate(outs, axis=0), dtype=np.float32)
    if _trace:
        return out, res
    return out
